# revision 1
# baseline (speedup 1.0000x reference)
"""Trainium2 Bass kernel for nn_CrossLayer (dense transformer layer).

Sharding: sequence-parallel over 8 cores (2 samples x 4 token-chunks of 512).
Each core computes its 512 token rows through CA -> SA -> FFN. K/V for all 16
heads are computed from each core's own rows and AllGather'd (bf16) across the
4 cores of its sample, once per attention block.

On-chip layout: activations feature-major [dim(128p x 8c), tok] so every
matmul contracts over partitions. RMSNorm partition-sums via ones-matmuls on
PE; RoPE rotate-half via a constant +-1 block matrix on PE; softmax
denominators via an appended ones column on V; exp without max subtraction
(scores are O(1): q/k are rms-normalized and /sqrt(d)).
"""

import math
import sys
import types

import numpy as np
import ml_dtypes

B, N, DIM, HID, H, D = 2, 2048, 1024, 4096, 16, 64
TOK = 512  # tokens per core
NCORES = 8
EPS = 1e-6
THETA = 10000.0
P = 128
KO = DIM // P  # 8 contraction chunks
HH = H // 2  # 8 head pairs
HC = HID // P  # 32 hidden chunks
TC = TOK // P  # 4 token chunks per core
NR = 4  # ranks per replica group
VW = D + 1  # v columns + ones column

BF = ml_dtypes.bfloat16

_cache = {}


def _lhsT_layout(W):
    """[K, M] -> [M//128, 128(K%128), K//128, 128(M%128)]: SBUF slices are
    matmul lhsT tiles [128, 128]."""
    K, M = W.shape
    return (
        W.reshape(K // P, P, M // P, P).transpose(2, 1, 0, 3).astype(BF).copy()
    )


def _rhs_layout(W):
    """[K, M] -> [128, K//128, M] rhs-style."""
    K, M = W.shape
    return W.reshape(K // P, P, M).transpose(1, 0, 2).astype(BF).copy()


def _featmajor(x):
    """[tok, dim] -> [128, dim//128, tok] float32."""
    return x.T.reshape(DIM // P, P, x.shape[0]).transpose(1, 0, 2).copy()


def _rope_tables(pos):
    """pos [TOK] int32 -> cos/sin [128, TOK] (2 heads stacked) bf16."""
    invf = 1.0 / (THETA ** (np.arange(0, D, 2, dtype=np.float64) / D))  # [32]
    ang = pos.astype(np.float64)[None, :] * invf[:, None]  # [32, TOK]
    c = np.cos(ang)
    s = np.sin(ang)
    c64 = np.concatenate([c, c], axis=0)  # [64, TOK]
    s64 = np.concatenate([s, s], axis=0)
    c128 = np.concatenate([c64, c64], axis=0).astype(BF)  # [128, TOK]
    s128 = np.concatenate([s64, s64], axis=0).astype(BF)
    return c128.copy(), s128.copy()


def _install_ntff_hook():
    try:
        from trn_agent_boot.trn_boot import _ntff_profile_via_ctypes
    except ImportError:
        return
    if "antenv.axon_hooks" in sys.modules:
        return
    try:
        hook = _ntff_profile_via_ctypes("/opt/axon/libaxon_pjrt.so")
    except OSError:
        return
    mod = types.ModuleType("antenv.axon_hooks")
    mod.get_axon_ntff_profile_hook = lambda: hook
    mod.set_axon_ntff_profile_hook = lambda h: None
    sys.modules["antenv.axon_hooks"] = mod
    import antenv

    antenv.axon_hooks = mod


def _split_multiwait(nc):
    """This walrus only supports one sync-wait on CTRL-encoded instructions
    (Drain/NoOp); hoist excess waits onto single-wait NoOps placed before."""
    from concourse import mybir

    n_split = 0
    for f in nc.m.functions:
        for bb in f.blocks:
            new = []
            changed = False
            for ins in bb.instructions:
                si = ins.sync_info
                if (
                    si is not None
                    and si.on_wait is not None
                    and len(si.on_wait) > 1
                ):
                    waits = list(si.on_wait)
                    keep, rest = waits[:1], waits[1:]
                    for k, w in enumerate(rest):
                        new.append(
                            mybir.InstNoOp(
                                name=f"{ins.name}-wsplit{k}",
                                engine=ins.engine,
                                sync_info=mybir.SyncInfo(
                                    on_wait=[w], on_update=[]
                                ),
                                bass_nofuse=True,
                            )
                        )
                    si.on_wait = keep
                    n_split += 1
                    changed = True
                new.append(ins)
            if changed:
                bb.instructions = new
    return n_split


def _build_bass():
    from contextlib import ExitStack

    import concourse.bass as bass
    import concourse.tile as tile
    from concourse import mybir

    f32 = mybir.dt.float32
    bf16 = mybir.dt.bfloat16
    AF = mybir.ActivationFunctionType

    nc = bass.Bass(num_devices=NCORES)

    def inp(name, shape, dt=bf16):
        return nc.dram_tensor(name, shape, dt, kind="ExternalInput")

    tgtT = inp("tgtT", [P, KO, TOK], f32)
    srcTb = inp("srcTb", [P, KO, TOK])
    cosq = inp("cosq", [P, TOK])
    sinq = inp("sinq", [P, TOK])
    coskca = inp("coskca", [P, TOK])
    sinkca = inp("sinkca", [P, TOK])
    caWq = inp("caWq", [HH, P, KO, P])
    caWk = inp("caWk", [HH, P, KO, P])
    caWv = inp("caWv", [P, KO, DIM])
    caWo = inp("caWo", [KO, P, KO, P])
    saWq = inp("saWq", [HH, P, KO, P])
    saWk = inp("saWk", [HH, P, KO, P])
    saWv = inp("saWv", [P, KO, DIM])
    saWo = inp("saWo", [KO, P, KO, P])
    W1i = inp("W1", [HC, P, KO, P])
    W3i = inp("W3", [HC, P, KO, P])
    W2i = inp("W2", [KO, P, HC, P])
    blk2 = inp("blk2", [P, 2])  # per-head ssq lhsT (block ones)
    mq_ca = inp("mq_ca", [2, P])  # rsqrt bcast lhsT with qn folded
    mk_ca = inp("mk_ca", [2, P])
    mq_sa = inp("mq_sa", [2, P])
    mk_sa = inp("mk_sa", [2, P])
    rotm = inp("rotm", [P, P])  # rotate-half (2-head block diag) lhsT
    ones_c = inp("ones_c", [P, 1])  # y-norm ssq lhsT
    ones_r128 = inp("ones_r128", [1, P])  # y-norm bcast lhsT

    outT = nc.dram_tensor("outT", [P, KO, TOK], f32, kind="ExternalOutput")

    groups = [[0, 1, 2, 3], [4, 5, 6, 7]]
    KWORDS = P * HH * TOK  # k bf16 words per rank
    VWORDS = P * TC * H * VW  # v bf16 words per rank

    with tile.TileContext(nc) as tc:
        ctx = ExitStack()
        with ctx:
            sing = ctx.enter_context(tc.tile_pool(name="sing", bufs=1))
            wpool = ctx.enter_context(tc.tile_pool(name="wpool", bufs=2))
            w2pool = ctx.enter_context(tc.tile_pool(name="w2pool", bufs=2))
            work = ctx.enter_context(tc.tile_pool(name="work", bufs=3))
            probp = ctx.enter_context(tc.tile_pool(name="probp", bufs=2))
            stat = ctx.enter_context(tc.tile_pool(name="stat", bufs=2))
            kvpool = ctx.enter_context(tc.tile_pool(name="kvpool", bufs=1))
            dram = ctx.enter_context(
                tc.tile_pool(name="dram", bufs=1, space="DRAM")
            )
            pp = ctx.enter_context(tc.tile_pool(name="pp", bufs=2, space="PSUM"))
            ps_s = ctx.enter_context(
                tc.tile_pool(name="ps_s", bufs=2, space="PSUM")
            )
            ps_x = ctx.enter_context(
                tc.tile_pool(name="ps_x", bufs=1, space="PSUM")
            )

            # ---- resident tiles
            resid = sing.tile([P, KO, TOK], f32)
            nc.sync.dma_start(resid[:], tgtT[:])
            srcT_sb = kvpool.tile([P, KO, TOK], bf16, tag="xT", name="srcT_sb")
            nc.sync.dma_start(srcT_sb[:], srcTb[:])
            cosq_sb = sing.tile([P, TOK], bf16)
            nc.sync.dma_start(cosq_sb[:], cosq[:])
            sinq_sb = sing.tile([P, TOK], bf16)
            nc.sync.dma_start(sinq_sb[:], sinq[:])
            coskca_sb = sing.tile([P, TOK], bf16)
            nc.sync.dma_start(coskca_sb[:], coskca[:])
            sinkca_sb = sing.tile([P, TOK], bf16)
            nc.sync.dma_start(sinkca_sb[:], sinkca[:])
            blk2_sb = sing.tile([P, 2], bf16)
            nc.sync.dma_start(blk2_sb[:], blk2[:])
            masks_sb = {}
            for name, t in (
                ("mq_ca", mq_ca),
                ("mk_ca", mk_ca),
                ("mq_sa", mq_sa),
                ("mk_sa", mk_sa),
            ):
                m = sing.tile([2, P], bf16, name=name)
                nc.sync.dma_start(m[:], t[:])
                masks_sb[name] = m
            rotm_sb = sing.tile([P, P], bf16)
            nc.sync.dma_start(rotm_sb[:], rotm[:])
            ones_c_sb = sing.tile([P, 1], bf16)
            nc.sync.dma_start(ones_c_sb[:], ones_c[:])
            ones_r128_sb = sing.tile([1, P], bf16)
            nc.sync.dma_start(ones_r128_sb[:], ones_r128[:])
            eps_sb = sing.tile([2, 1], mybir.dt.float32)
            nc.vector.memset(eps_sb[:], float(EPS))

            def norm_rope_one(psum_q, mask_sb, cos_sb, sin_sb, dst):
                """psum_q [128(2 heads), TOK] f32 -> dst bf16: rms-normed,
                qn-scaled, roped."""
                raw = stat.tile([P, TOK], f32, tag="raw", name="raw")
                nc.vector.tensor_copy(raw[:], psum_q[:])
                sq = work.tile([P, TOK], bf16, tag="ysq", name="sq")
                nc.vector.tensor_mul(sq[:], raw[:], raw[:])
                ssq = pp.tile([2, TOK], f32, tag="pp", name="ssq")
                nc.tensor.matmul(ssq[:], blk2_sb[:], sq[:], start=True, stop=True)
                # rsqrt(mean+eps) = exp(-0.5*ln(mean+eps)); Ln/Exp share one
                # ACT table set (natural_log_exp) with the attention exps
                lnt = stat.tile([2, TOK], f32, tag="lnt", name="lnt")
                nc.scalar.activation(
                    lnt[:], ssq[:], AF.Ln, bias=eps_sb[:], scale=1.0 / D
                )
                rs = stat.tile([2, TOK], bf16, tag="rs", name="rs")
                nc.scalar.activation(rs[:], lnt[:], AF.Exp, scale=-0.5)
                bc = pp.tile([P, TOK], f32, tag="pp", name="bc")
                nc.tensor.matmul(bc[:], mask_sb[:], rs[:], start=True, stop=True)
                v1 = stat.tile([P, TOK], bf16, tag="v1", name="v1")
                nc.vector.tensor_mul(v1[:], raw[:], bc[:])
                rot_ps = pp.tile([P, TOK], f32, tag="pp", name="rot_ps")
                nc.tensor.matmul(
                    rot_ps[:], rotm_sb[:], v1[:], start=True, stop=True
                )
                rot = stat.tile([P, TOK], bf16, tag="rot", name="rot")
                nc.scalar.copy(rot[:], rot_ps[:])
                t1 = stat.tile([P, TOK], bf16, tag="t1", name="t1")
                nc.vector.tensor_mul(t1[:], v1[:], cos_sb[:])
                nc.vector.tensor_mul(dst, rot[:], sin_sb[:])
                nc.vector.tensor_add(dst, t1[:], dst)

            def rmsnorm_feat(src_f32, dst_bf16):
                """Feature-major RMSNorm: dst = src * rsqrt(mean(src^2))."""
                ssq = pp.tile([1, TOK], f32, tag="pp", name="yssq")
                for c in range(KO):
                    sq = work.tile([P, TOK], bf16, tag="ysq", name="ynsq")
                    nc.vector.tensor_mul(sq[:], src_f32[:, c], src_f32[:, c])
                    nc.tensor.matmul(
                        ssq[:],
                        ones_c_sb[:],
                        sq[:],
                        start=(c == 0),
                        stop=(c == KO - 1),
                    )
                lnt = stat.tile([1, TOK], f32, tag="lnt", name="ylnt")
                nc.scalar.activation(
                    lnt[:], ssq[:], AF.Ln, bias=eps_sb[:1], scale=1.0 / DIM
                )
                rs = stat.tile([1, TOK], bf16, tag="rs", name="yrs")
                nc.scalar.activation(rs[:], lnt[:], AF.Exp, scale=-0.5)
                bc = pp.tile([P, TOK], f32, tag="pp", name="ybc")
                nc.tensor.matmul(
                    bc[:], ones_r128_sb[:], rs[:], start=True, stop=True
                )
                for c in range(KO):
                    nc.vector.tensor_mul(dst_bf16[:, c], src_f32[:, c], bc[:])

            def attention_block(y_sb, kvsrc_sb, Wq_t, Wk_t, Wv_t, Wo_t,
                                mq, mk, cosk, sink):
                """One attention block; y_sb bf16 [P,KO,TOK] is the q-side
                input, kvsrc_sb the kv-side input. Adds Wo output into resid."""
                # --- k projection + norm/rope from my rows
                k_mine = kvpool.tile([P, HH, TOK], bf16, tag="kq", name="k_mine")
                for g in range(2):  # stream Wk in halves
                    wk = wpool.tile([P, 4, KO, P], bf16, tag="w1m", name="wk")
                    nc.sync.dma_start(
                        wk[:],
                        Wk_t[g * 4 : (g + 1) * 4].rearrange(
                            "g p ko m -> p g ko m"
                        ),
                    )
                    for j in range(4):
                        hh = g * 4 + j
                        pk = pp.tile([P, TOK], f32, tag="pp", name="pk")
                        for c in range(KO):
                            nc.tensor.matmul(
                                pk[:],
                                wk[:, j, c],
                                kvsrc_sb[:, c],
                                start=(c == 0),
                                stop=(c == KO - 1),
                            )
                        norm_rope_one(pk, mk, cosk, sink, k_mine[:, hh])

                # --- v projection (token-major, with ones column)
                v_mine = kvpool.tile(
                    [P, TC, H, VW], bf16, tag="vm", name="v_mine"
                )
                nc.vector.memset(v_mine[:, :, :, D : D + 1], 1.0)
                for nh in range(2):
                    wv = wpool.tile([P, KO, TOK], bf16, tag="w1m", name="wv")
                    nc.sync.dma_start(
                        wv[:], Wv_t[:, :, nh * TOK : (nh + 1) * TOK]
                    )
                    for t4 in range(TC):
                        pv = pp.tile([P, TOK], f32, tag="pp", name="pv")
                        for c in range(KO):
                            nc.tensor.matmul(
                                pv[:],
                                kvsrc_sb[:, c, t4 * P : (t4 + 1) * P],
                                wv[:, c],
                                start=(c == 0),
                                stop=(c == KO - 1),
                            )
                        nc.vector.tensor_copy(
                            v_mine[:, t4, nh * 8 : (nh + 1) * 8, 0:D],
                            pv[:].rearrange("p (h d) -> p h d", d=D),
                        )

                # --- allgather k/v across my sample's 4 cores
                kv_in = dram.tile([KWORDS + VWORDS], bf16, tag="kv_in")
                nc.sync.dma_start(
                    kv_in[:KWORDS].rearrange(
                        "(p h t) -> p h t", p=P, h=HH, t=TOK
                    ),
                    k_mine[:],
                )
                nc.sync.dma_start(
                    kv_in[KWORDS:].rearrange(
                        "(p a b c) -> p a b c", p=P, a=TC, b=H, c=VW
                    ),
                    v_mine[:],
                )
                kv_out = dram.tile([NR, KWORDS + VWORDS], bf16, tag="kv_out")
                nc.gpsimd.collective_compute(
                    "AllGather",
                    mybir.AluOpType.bypass,
                    replica_groups=groups,
                    ins=[kv_in.opt()],
                    outs=[kv_out.opt()],
                )
                k_full = kvpool.tile(
                    [P, HH, NR, TOK], bf16, tag="k_full", name="k_full"
                )
                v_full = kvpool.tile(
                    [P, NR, TC, H, VW], bf16, tag="v_full", name="v_full"
                )
                for r in range(NR):
                    nc.sync.dma_start(
                        k_full[:, :, r],
                        kv_out[r, :KWORDS].rearrange(
                            "(p h t) -> p h t", p=P, h=HH, t=TOK
                        ),
                    )
                    nc.sync.dma_start(
                        v_full[:, r],
                        kv_out[r, KWORDS:].rearrange(
                            "(p a b c) -> p a b c", p=P, a=TC, b=H, c=VW
                        ),
                    )

                # --- q projection + norm + rope (overlaps the collective)
                q_sb = kvpool.tile([P, HH, TOK], bf16, tag="kq", name="q_sb")
                for g in range(2):
                    wq = wpool.tile([P, 4, KO, P], bf16, tag="w1m", name="wq")
                    nc.sync.dma_start(
                        wq[:],
                        Wq_t[g * 4 : (g + 1) * 4].rearrange(
                            "g p ko m -> p g ko m"
                        ),
                    )
                    for j in range(4):
                        hh = g * 4 + j
                        pq = pp.tile([P, TOK], f32, tag="pp", name="pq")
                        for c in range(KO):
                            nc.tensor.matmul(
                                pq[:],
                                wq[:, j, c],
                                y_sb[:, c],
                                start=(c == 0),
                                stop=(c == KO - 1),
                            )
                        norm_rope_one(pq, mq, cosq_sb, sinq_sb, q_sb[:, hh])

                # --- attention: 2 heads share one exp; denominators ride in
                # row 64 of the px accumulators (ones column of v)
                xT = kvpool.tile([P, HH, TOK], bf16, tag="xT", name="xT")
                dens = kvpool.tile([D + 1, H, TOK], bf16, tag="dens", name="dens")
                for hh in range(HH):
                    px = [
                        ps_x.tile([VW, TOK], f32, tag=f"px{i}", name=f"px{i}")
                        for i in range(2)
                    ]
                    for kc in range(H):  # 16 k-chunks of 128 tokens
                        r, tcl = kc // TC, kc % TC
                        ps = ps_s.tile([P, 2 * TOK], f32, tag="ps", name="ps")
                        for i in range(2):
                            off = i * D
                            nc.tensor.matmul(
                                ps[:, i * TOK : (i + 1) * TOK],
                                k_full[
                                    off : off + D,
                                    hh,
                                    r,
                                    tcl * P : (tcl + 1) * P,
                                ],
                                q_sb[off : off + D, hh],
                                start=True,
                                stop=True,
                            )
                        prob = probp.tile(
                            [P, 2 * TOK], bf16, tag="prob", name="prob"
                        )
                        nc.scalar.activation(
                            prob[:], ps[:], AF.Exp, scale=1.0 / math.sqrt(D)
                        )
                        for i in range(2):
                            h = hh * 2 + i
                            nc.tensor.matmul(
                                px[i][:],
                                v_full[:, r, tcl, h],
                                prob[:, i * TOK : (i + 1) * TOK],
                                start=(kc == 0),
                                stop=(kc == H - 1),
                            )
                    for i in range(2):
                        h = hh * 2 + i
                        # denom row lives on partition 64; keep it there
                        nc.vector.tensor_copy(
                            dens[D : D + 1, h], px[i][D : D + 1]
                        )
                        # 64-channel copy may retarget the other half-window
                        nc.vector.tensor_copy(
                            xT[i * D : (i + 1) * D, hh], px[i][0:D]
                        )

                # --- softmax denominators: one reciprocal, broadcast via DRAM
                dflat = dens[D : D + 1].rearrange("o h t -> o (h t)")
                nc.scalar.activation(dflat, dflat, AF.Ln)
                nc.scalar.activation(dflat, dflat, AF.Exp, scale=-1.0)
                db = dram.tile([H * TOK], bf16, tag="db")
                nc.sync.dma_start(
                    db[:].rearrange("(o h t) -> o h t", o=1, h=H),
                    dens[D : D + 1],
                )
                rec_bc = kvpool.tile(
                    [P, HH, TOK], bf16, tag="vm", name="rec_bc"
                )
                for i in range(2):
                    src = bass.AP(
                        tensor=db.tensor,
                        offset=db.offset + i * TOK,
                        ap=[[0, D], [2 * TOK, HH], [1, TOK]],
                    )
                    nc.sync.dma_start(rec_bc[i * D : (i + 1) * D], src)
                for hh in range(HH):
                    nc.vector.tensor_mul(
                        xT[:, hh], xT[:, hh], rec_bc[:, hh]
                    )

                # --- Wo projection, accumulate into resid
                for g in range(2):
                    wo = wpool.tile([P, 4, KO, P], bf16, tag="w1m", name="wo")
                    nc.sync.dma_start(
                        wo[:],
                        Wo_t[g * 4 : (g + 1) * 4].rearrange(
                            "g p ko m -> p g ko m"
                        ),
                    )
                    for j in range(4):
                        oc = g * 4 + j
                        po = pp.tile([P, TOK], f32, tag="pp", name="po")
                        for c in range(KO):
                            nc.tensor.matmul(
                                po[:],
                                wo[:, j, c],
                                xT[:, c],
                                start=(c == 0),
                                stop=(c == KO - 1),
                            )
                        nc.vector.tensor_add(resid[:, oc], resid[:, oc], po[:])

            # ================= cross-attention =================
            yT = sing.tile([P, KO, TOK], bf16, name="yT")
            rmsnorm_feat(resid, yT)
            attention_block(
                yT, srcT_sb, caWq, caWk, caWv, caWo,
                masks_sb["mq_ca"], masks_sb["mk_ca"], coskca_sb, sinkca_sb,
            )

            # ================= self-attention =================
            rmsnorm_feat(resid, yT)
            attention_block(
                yT, yT, saWq, saWk, saWv, saWo,
                masks_sb["mq_sa"], masks_sb["mk_sa"], cosq_sb, sinq_sb,
            )

            # ================= FFN =================
            rmsnorm_feat(resid, yT)
            hT = kvpool.tile([P, HC, TOK], bf16, tag="k_full", name="hT")
            for g in range(8):  # stream W1/W3 in eighths
                w1 = wpool.tile([P, 4, KO, P], bf16, tag="w1m", name="w1")
                nc.sync.dma_start(
                    w1[:],
                    W1i[g * 4 : (g + 1) * 4].rearrange("g p ko m -> p g ko m"),
                )
                w3 = wpool.tile([P, 4, KO, P], bf16, tag="w1m", name="w3")
                nc.sync.dma_start(
                    w3[:],
                    W3i[g * 4 : (g + 1) * 4].rearrange("g p ko m -> p g ko m"),
                )
                for j in range(4):
                    hc = g * 4 + j
                    p1 = pp.tile([P, TOK], f32, tag="pp", name="p1")
                    for c in range(KO):
                        nc.tensor.matmul(
                            p1[:], w1[:, j, c], yT[:, c],
                            start=(c == 0), stop=(c == KO - 1),
                        )
                    p3 = pp.tile([P, TOK], f32, tag="pp", name="p3")
                    for c in range(KO):
                        nc.tensor.matmul(
                            p3[:], w3[:, j, c], yT[:, c],
                            start=(c == 0), stop=(c == KO - 1),
                        )
                    s1 = stat.tile([P, TOK], f32, tag="raw", name="s1")
                    nc.scalar.activation(s1[:], p1[:], AF.Silu)
                    nc.vector.tensor_mul(hT[:, hc], s1[:], p3[:])
            for oc in range(KO):
                w2 = w2pool.tile([P, HC, P], bf16, tag="w2", name="w2")
                nc.sync.dma_start(w2[:], W2i[oc])
                po = pp.tile([P, TOK], f32, tag="pp", name="po2")
                for hc in range(HC):
                    nc.tensor.matmul(
                        po[:], w2[:, hc], hT[:, hc],
                        start=(hc == 0), stop=(hc == HC - 1),
                    )
                nc.vector.tensor_add(resid[:, oc], resid[:, oc], po[:])

            nc.sync.dma_start(outT[:], resid[:])

    _split_multiwait(nc)
    return nc


def _prep_inputs(inputs):
    """Full problem inputs -> list of 8 per-core in_maps."""
    tgt = np.asarray(inputs["tgt"], np.float32)
    src = np.asarray(inputs["src"], np.float32)
    tgt_pos = np.asarray(inputs["tgt_pos"], np.int32)
    src_pos = np.asarray(inputs["src_pos"], np.int32)

    pre_ca_w = np.asarray(inputs["pre_ca_w"], np.float32)
    pre_sa_w = np.asarray(inputs["pre_sa_w"], np.float32)
    pre_ffn_w = np.asarray(inputs["pre_ffn_w"], np.float32)

    def fold(Wname, w):
        return np.asarray(inputs[Wname], np.float32) * w[:, None]

    ca_Wq = fold("ca_Wq", pre_ca_w)
    ca_Wkv = np.asarray(inputs["ca_Wkv"], np.float32)
    ca_Wk, ca_Wv = ca_Wkv[:, :DIM], ca_Wkv[:, DIM:]
    ca_Wo = np.asarray(inputs["ca_Wo"], np.float32)
    sa_Wq = fold("sa_Wq", pre_sa_w)
    sa_Wkv = fold("sa_Wkv", pre_sa_w)
    sa_Wk, sa_Wv = sa_Wkv[:, :DIM], sa_Wkv[:, DIM:]
    sa_Wo = np.asarray(inputs["sa_Wo"], np.float32)
    W1 = fold("W1", pre_ffn_w)
    W3 = fold("W3", pre_ffn_w)
    W2 = np.asarray(inputs["W2"], np.float32)

    shared = {
        "caWq": _lhsT_layout(ca_Wq),
        "caWk": _lhsT_layout(ca_Wk),
        "caWv": _rhs_layout(ca_Wv),
        "caWo": _lhsT_layout(ca_Wo),
        "saWq": _lhsT_layout(sa_Wq),
        "saWk": _lhsT_layout(sa_Wk),
        "saWv": _rhs_layout(sa_Wv),
        "saWo": _lhsT_layout(sa_Wo),
        "W1": _lhsT_layout(W1),
        "W3": _lhsT_layout(W3),
        "W2": _lhsT_layout(W2),
    }

    blk2 = np.zeros((P, 2), BF)
    blk2[:D, 0] = 1
    blk2[D:, 1] = 1
    shared["blk2"] = blk2

    def head_mask(w):  # [2, 128] with per-head norm weight
        m = np.zeros((2, P), np.float32)
        m[0, :D] = w
        m[1, D:] = w
        return m.astype(BF).copy()

    shared["mq_ca"] = head_mask(np.asarray(inputs["ca_qn"], np.float32))
    shared["mk_ca"] = head_mask(np.asarray(inputs["ca_kn"], np.float32))
    shared["mq_sa"] = head_mask(np.asarray(inputs["sa_qn"], np.float32))
    shared["mk_sa"] = head_mask(np.asarray(inputs["sa_kn"], np.float32))

    r64 = np.zeros((D, D), np.float32)
    half = D // 2
    for j in range(half):
        r64[j, j + half] = -1.0  # rot[j] = -x[j+32]
        r64[j + half, j] = 1.0  # rot[j+32] = x[j]
    rt = r64.T  # lhsT (matmul computes lhsT.T @ rhs)
    rotm = np.zeros((P, P), np.float32)
    rotm[:D, :D] = rt
    rotm[D:, D:] = rt
    shared["rotm"] = rotm.astype(BF).copy()

    shared["ones_c"] = np.ones((P, 1), BF)
    shared["ones_r128"] = np.ones((1, P), BF)

    in_maps = []
    for c in range(NCORES):
        s, part = c // NR, c % NR
        rows = slice(part * TOK, (part + 1) * TOK)
        m = dict(shared)
        m["tgtT"] = _featmajor(tgt[s, rows])
        m["srcTb"] = _featmajor(src[s, rows]).astype(BF)
        cq, sq_ = _rope_tables(tgt_pos[s, rows])
        ck, sk = _rope_tables(src_pos[s, rows])
        m["cosq"], m["sinq"] = cq, sq_
        m["coskca"], m["sinkca"] = ck, sk
        in_maps.append(m)
    return in_maps


def _get_nc():
    if "nc" not in _cache:
        _cache["nc"] = _build_bass()
    return _cache["nc"]


def run(inputs, trace=False):
    """Run on 8 cores; returns (full_output, exec_time_ns_or_None)."""
    if trace:
        _install_ntff_hook()
    from concourse.bass_utils import run_bass_kernel_spmd

    in_maps = _prep_inputs(inputs)
    nc = _get_nc()
    res = run_bass_kernel_spmd(
        nc, in_maps, core_ids=list(range(NCORES)), trace=trace
    )
    out = np.empty((B, N, DIM), np.float32)
    for c in range(NCORES):
        s, part = c // NR, c % NR
        arr = np.asarray(res.results[c]["outT"])  # [128, 8, TOK]
        rows = slice(part * TOK, (part + 1) * TOK)
        out[s, rows] = np.transpose(arr, (2, 1, 0)).reshape(TOK, DIM)
    return out, res.exec_time_ns


def kernel(**inputs):
    out, _ = run(inputs, trace=False)
    return out



# revision 6
# speedup vs baseline: 1.6183x; 1.6183x over previous
"""Trainium2 Bass kernel for nn_CrossLayer (dense transformer layer).

Sharding: sequence-parallel over 8 cores (2 samples x 4 token-chunks of 512).
Each core computes its 512 token rows through CA -> SA -> FFN. K/V (fp8,
all 16 heads) are computed from each core's own rows and AllGather'd across
the 4 cores of its sample in two per-head-half collectives so the gathers
hide under projection/attention compute.

Matmuls run fp8 (e4m3) with DoubleRow perf mode (2 k-subtiles per
instruction, 2x column rate). Weights are prescaled x32 to dodge fp8
subnormals; the descale is folded into epilogues: rmsnorm cancels it for
Q/K, the softmax-denominator reciprocal absorbs it for V (exp bias), and
scalar_tensor_tensor fuses (x 1/32 + resid) for Wo/W2, Silu(scale=1/32)
for W1, (x 1/32) * silu for W3.

On-chip layout: activations feature-major [dim(128p x 8c), tok] so every
matmul contracts over partitions. RMSNorm partition-sums via ones-matmuls on
PE; RoPE rotate-half via a constant +-1 block matrix on PE; softmax
denominators via an appended ones column on V; exp without max subtraction
(scores are O(1): q/k are rms-normalized and /sqrt(d)).
"""

import math
import sys
import types

import numpy as np
import ml_dtypes

B, N, DIM, HID, H, D = 2, 2048, 1024, 4096, 16, 64
TOK = 512  # tokens per core
NCORES = 8
EPS = 1e-6
THETA = 10000.0
P = 128
KO = DIM // P  # 8 contraction chunks
HH = H // 2  # 8 head pairs
HC = HID // P  # 32 hidden chunks
TC = TOK // P  # 4 token chunks per core
NR = 4  # ranks per replica group
VW = D + 1  # v columns + ones column
WS = 32.0  # fp8 weight prescale
LN_WS = math.log(WS)

BF = ml_dtypes.bfloat16
F8 = ml_dtypes.float8_e4m3

KB = P * 4 * TOK  # k fp8 bytes per rank per half
VB = P * TC * 8 * VW  # v fp8 bytes per rank per half

_cache = {}


def _lhsT_f8(W):
    """[K, M] -> [M//128, 128(K%128), K//128, 128(M%128)] fp8, x32."""
    K, M = W.shape
    a = (W * WS).reshape(K // P, P, M // P, P).transpose(2, 1, 0, 3)
    return np.clip(a, -240, 240).astype(F8).copy()


def _rhs_f8(W):
    """[K, M] -> [128, K//128, M] rhs-style fp8, x32."""
    K, M = W.shape
    a = (W * WS).reshape(K // P, P, M).transpose(1, 0, 2)
    return np.clip(a, -240, 240).astype(F8).copy()


def _featmajor(x):
    """[tok, dim] -> [128, dim//128, tok] float32."""
    return x.T.reshape(DIM // P, P, x.shape[0]).transpose(1, 0, 2).copy()


def _rope_tables(pos):
    """pos [TOK] int32 -> cos/sin [128, TOK] (2 heads stacked) bf16."""
    invf = 1.0 / (THETA ** (np.arange(0, D, 2, dtype=np.float64) / D))  # [32]
    ang = pos.astype(np.float64)[None, :] * invf[:, None]  # [32, TOK]
    c = np.cos(ang)
    s = np.sin(ang)
    c64 = np.concatenate([c, c], axis=0)  # [64, TOK]
    s64 = np.concatenate([s, s], axis=0)
    c128 = np.concatenate([c64, c64], axis=0).astype(BF)  # [128, TOK]
    s128 = np.concatenate([s64, s64], axis=0).astype(BF)
    return c128.copy(), s128.copy()


def _install_ntff_hook():
    try:
        from trn_agent_boot.trn_boot import _ntff_profile_via_ctypes
    except ImportError:
        return
    if "antenv.axon_hooks" in sys.modules:
        return
    try:
        hook = _ntff_profile_via_ctypes("/opt/axon/libaxon_pjrt.so")
    except OSError:
        return
    mod = types.ModuleType("antenv.axon_hooks")
    mod.get_axon_ntff_profile_hook = lambda: hook
    mod.set_axon_ntff_profile_hook = lambda h: None
    sys.modules["antenv.axon_hooks"] = mod
    import antenv

    antenv.axon_hooks = mod


def _split_multiwait(nc):
    """This walrus only supports one sync-wait on CTRL-encoded instructions
    (Drain/NoOp); hoist excess waits onto single-wait NoOps placed before."""
    from concourse import mybir

    n_split = 0
    for f in nc.m.functions:
        for bb in f.blocks:
            new = []
            changed = False
            for ins in bb.instructions:
                si = ins.sync_info
                if (
                    si is not None
                    and si.on_wait is not None
                    and len(si.on_wait) > 1
                ):
                    waits = list(si.on_wait)
                    keep, rest = waits[:1], waits[1:]
                    for k, w in enumerate(rest):
                        new.append(
                            mybir.InstNoOp(
                                name=f"{ins.name}-wsplit{k}",
                                engine=ins.engine,
                                sync_info=mybir.SyncInfo(
                                    on_wait=[w], on_update=[]
                                ),
                                bass_nofuse=True,
                            )
                        )
                    si.on_wait = keep
                    n_split += 1
                    changed = True
                new.append(ins)
            if changed:
                bb.instructions = new
    return n_split


def _build_bass():
    from contextlib import ExitStack

    import concourse.bass as bass
    import concourse.tile as tile
    from concourse import mybir

    f32 = mybir.dt.float32
    bf16 = mybir.dt.bfloat16
    f8 = mybir.dt.float8e4
    AF = mybir.ActivationFunctionType
    DRM = mybir.MatmulPerfMode.DoubleRow
    MUL = mybir.AluOpType.mult
    ADD = mybir.AluOpType.add

    nc = bass.Bass(num_devices=NCORES)

    def inp(name, shape, dt=bf16):
        return nc.dram_tensor(name, shape, dt, kind="ExternalInput")

    tgtT = inp("tgtT", [P, KO, TOK], f32)
    srcTb = inp("srcTb", [P, KO, TOK], f8)
    cosq = inp("cosq", [P, TOK])
    sinq = inp("sinq", [P, TOK])
    coskca = inp("coskca", [P, TOK])
    sinkca = inp("sinkca", [P, TOK])
    caWq = inp("caWq", [HH, P, KO, P], f8)
    caWk = inp("caWk", [HH, P, KO, P], f8)
    caWv = inp("caWv", [P, KO, DIM], f8)
    caWo = inp("caWo", [KO, P, KO, P], f8)
    saWq = inp("saWq", [HH, P, KO, P], f8)
    saWk = inp("saWk", [HH, P, KO, P], f8)
    saWv = inp("saWv", [P, KO, DIM], f8)
    saWo = inp("saWo", [KO, P, KO, P], f8)
    W1i = inp("W1", [HC, P, KO, P], f8)
    W3i = inp("W3", [HC, P, KO, P], f8)
    W2i = inp("W2", [KO, P, HC, P], f8)
    blk2 = inp("blk2", [P, 2])  # per-head ssq lhsT (block ones)
    mq_ca = inp("mq_ca", [2, P])  # rsqrt bcast lhsT with qn folded
    mk_ca = inp("mk_ca", [2, P])
    mq_sa = inp("mq_sa", [2, P])
    mk_sa = inp("mk_sa", [2, P])
    rotm = inp("rotm", [P, P])  # rotate-half (2-head block diag) lhsT
    ones_c = inp("ones_c", [P, 1])  # y-norm ssq lhsT
    ones_r128 = inp("ones_r128", [1, P])  # y-norm bcast lhsT
    selden = inp("selden", [H, HH, P])  # denominator bcast lhsT masks

    outT = nc.dram_tensor("outT", [P, KO, TOK], f32, kind="ExternalOutput")

    groups = [[0, 1, 2, 3], [4, 5, 6, 7]]

    with tile.TileContext(nc) as tc:
        ctx = ExitStack()
        with ctx:
            sing = ctx.enter_context(tc.tile_pool(name="sing", bufs=1))
            wpool = ctx.enter_context(tc.tile_pool(name="wpool", bufs=4))
            w2pool = ctx.enter_context(tc.tile_pool(name="w2pool", bufs=2))
            work = ctx.enter_context(tc.tile_pool(name="work", bufs=3))
            probp = ctx.enter_context(tc.tile_pool(name="probp", bufs=2))
            stat = ctx.enter_context(tc.tile_pool(name="stat", bufs=2))
            kvpool = ctx.enter_context(tc.tile_pool(name="kvpool", bufs=1))
            dram = ctx.enter_context(
                tc.tile_pool(name="dram", bufs=1, space="DRAM")
            )
            pp = ctx.enter_context(tc.tile_pool(name="pp", bufs=2, space="PSUM"))
            ps_s = ctx.enter_context(
                tc.tile_pool(name="ps_s", bufs=2, space="PSUM")
            )
            ps_x = ctx.enter_context(
                tc.tile_pool(name="ps_x", bufs=1, space="PSUM")
            )

            # ---- resident tiles
            resid = sing.tile([P, KO, TOK], f32)
            nc.sync.dma_start(resid[:], tgtT[:])
            srcT_sb = kvpool.tile([P, KO, TOK], f8, tag="xT", name="srcT_sb")
            nc.sync.dma_start(srcT_sb[:], srcTb[:])
            cosq_sb = sing.tile([P, TOK], bf16)
            nc.sync.dma_start(cosq_sb[:], cosq[:])
            sinq_sb = sing.tile([P, TOK], bf16)
            nc.sync.dma_start(sinq_sb[:], sinq[:])
            coskca_sb = sing.tile([P, TOK], bf16)
            nc.sync.dma_start(coskca_sb[:], coskca[:])
            sinkca_sb = sing.tile([P, TOK], bf16)
            nc.sync.dma_start(sinkca_sb[:], sinkca[:])
            blk2_sb = sing.tile([P, 2], bf16)
            nc.sync.dma_start(blk2_sb[:], blk2[:])
            masks_sb = {}
            for name, t in (
                ("mq_ca", mq_ca),
                ("mk_ca", mk_ca),
                ("mq_sa", mq_sa),
                ("mk_sa", mk_sa),
            ):
                m = sing.tile([2, P], bf16, name=name)
                nc.sync.dma_start(m[:], t[:])
                masks_sb[name] = m
            rotm_sb = sing.tile([P, P], bf16)
            nc.sync.dma_start(rotm_sb[:], rotm[:])
            ones_c_sb = sing.tile([P, 1], bf16)
            nc.sync.dma_start(ones_c_sb[:], ones_c[:])
            ones_r128_sb = sing.tile([1, P], bf16)
            nc.sync.dma_start(ones_r128_sb[:], ones_r128[:])
            selden_sb = sing.tile([H, HH, P], bf16)
            nc.sync.dma_start(selden_sb[:], selden[:])
            eps_sb = sing.tile([2, 1], mybir.dt.float32)
            nc.vector.memset(eps_sb[:], float(EPS))
            lnws_sb = sing.tile([H, 1], mybir.dt.float32)
            nc.vector.memset(lnws_sb[:], -LN_WS)

            def dr_chain(out_ps, lhs_fn, rhs_fn):
                """KO-contraction via 4 DoubleRow fp8 matmuls."""
                for c2 in range(KO // 2):
                    nc.tensor.matmul(
                        out_ps,
                        lhs_fn(2 * c2),
                        rhs_fn(2 * c2),
                        start=(c2 == 0),
                        stop=(c2 == KO // 2 - 1),
                        perf_mode=DRM,
                    )

            def norm_rope_one(psum_q, mask_sb, cos_sb, sin_sb, dst):
                """psum_q [128(2 heads), TOK] f32 -> dst: rms-normed,
                qn-scaled, roped."""
                raw = stat.tile([P, TOK], f32, tag="raw", name="raw")
                nc.vector.tensor_copy(raw[:], psum_q[:])
                sq = work.tile([P, TOK], bf16, tag="ysq", name="sq")
                nc.vector.tensor_mul(sq[:], raw[:], raw[:])
                ssq = pp.tile([2, TOK], f32, tag="pp", name="ssq")
                nc.tensor.matmul(ssq[:], blk2_sb[:], sq[:], start=True, stop=True)
                # rsqrt(mean+eps) = exp(-0.5*ln(mean+eps)); Ln/Exp share one
                # ACT table set (natural_log_exp) with the attention exps
                lnt = stat.tile([2, TOK], f32, tag="lnt", name="lnt")
                nc.scalar.activation(
                    lnt[:], ssq[:], AF.Ln, bias=eps_sb[:], scale=1.0 / D
                )
                rs = stat.tile([2, TOK], bf16, tag="rs", name="rs")
                nc.scalar.activation(rs[:], lnt[:], AF.Exp, scale=-0.5)
                bc = pp.tile([P, TOK], f32, tag="pp", name="bc")
                nc.tensor.matmul(bc[:], mask_sb[:], rs[:], start=True, stop=True)
                v1 = stat.tile([P, TOK], bf16, tag="v1", name="v1")
                nc.vector.tensor_mul(v1[:], raw[:], bc[:])
                rot_ps = pp.tile([P, TOK], f32, tag="pp", name="rot_ps")
                nc.tensor.matmul(
                    rot_ps[:], rotm_sb[:], v1[:], start=True, stop=True
                )
                rot = stat.tile([P, TOK], bf16, tag="rot", name="rot")
                nc.vector.tensor_copy(rot[:], rot_ps[:])
                t1 = stat.tile([P, TOK], bf16, tag="t1", name="t1")
                nc.vector.tensor_mul(t1[:], v1[:], cos_sb[:])
                t2 = stat.tile([P, TOK], bf16, tag="t2", name="t2")
                nc.vector.tensor_mul(t2[:], rot[:], sin_sb[:])
                nc.vector.tensor_add(dst, t1[:], t2[:])

            def rmsnorm_feat(src_f32, dst_f8):
                """Feature-major RMSNorm: dst = src * rsqrt(mean(src^2))."""
                ssq = pp.tile([1, TOK], f32, tag="pp", name="yssq")
                for c in range(KO):
                    sq = work.tile([P, TOK], bf16, tag="ysq", name="ynsq")
                    nc.vector.tensor_mul(sq[:], src_f32[:, c], src_f32[:, c])
                    nc.tensor.matmul(
                        ssq[:],
                        ones_c_sb[:],
                        sq[:],
                        start=(c == 0),
                        stop=(c == KO - 1),
                    )
                lnt = stat.tile([1, TOK], f32, tag="lnt", name="ylnt")
                nc.scalar.activation(
                    lnt[:], ssq[:], AF.Ln, bias=eps_sb[:1], scale=1.0 / DIM
                )
                rs = stat.tile([1, TOK], bf16, tag="rs", name="yrs")
                nc.scalar.activation(rs[:], lnt[:], AF.Exp, scale=-0.5)
                bc = pp.tile([P, TOK], f32, tag="pp", name="ybc")
                nc.tensor.matmul(
                    bc[:], ones_r128_sb[:], rs[:], start=True, stop=True
                )
                for c in range(KO):
                    nc.vector.tensor_mul(dst_f8[:, c], src_f32[:, c], bc[:])

            def attention_block(kvsrc_sb, Wq_t, Wk_t, Wv_t, Wo_t,
                                mq, mk, cosk, sink, mid):
                """One attention block; kvsrc_sb fp8 [P,KO,TOK] is the kv-side
                input. mid() emits the q-side rmsnorm (overlaps the gathers)
                and returns y_sb. Adds Wo output into resid."""
                k_full = {}
                v_full = {}
                for half in range(2):
                    # --- k projection + norm/rope (head pairs half*4..+4)
                    k_mine = kvpool.tile(
                        [P, 4, TOK], f8, tag=f"km{half}", name=f"k_mine{half}"
                    )
                    wk = wpool.tile([P, 4, KO, P], f8, tag="w1m", name="wk")
                    nc.sync.dma_start(
                        wk[:],
                        Wk_t[half * 4 : (half + 1) * 4].rearrange(
                            "g p ko m -> p g ko m"
                        ),
                    )
                    for j in range(4):
                        pk = pp.tile([P, TOK], f32, tag="pp", name="pk")
                        dr_chain(
                            pk[:],
                            lambda c, j=j: wk[:, j, c : c + 2, :],
                            lambda c: kvsrc_sb[:, c : c + 2, :],
                        )
                        norm_rope_one(pk, mk, cosk, sink, k_mine[:, j])

                    # --- v projection (token-major, with ones column)
                    v_mine = kvpool.tile(
                        [P, TC, 8, VW], f8, tag=f"vm{half}",
                        name=f"v_mine{half}",
                    )
                    nc.vector.memset(v_mine[:, :, :, D : D + 1], 1.0)
                    wv = wpool.tile([P, KO, TOK], f8, tag="wv", name="wv")
                    nc.sync.dma_start(
                        wv[:], Wv_t[:, :, half * TOK : (half + 1) * TOK]
                    )
                    for t4 in range(TC):
                        pv = pp.tile([P, TOK], f32, tag="pp", name="pv")
                        dr_chain(
                            pv[:],
                            lambda c, t4=t4: kvsrc_sb[
                                :, c : c + 2, t4 * P : (t4 + 1) * P
                            ],
                            lambda c: wv[:, c : c + 2, :],
                        )
                        nc.vector.tensor_copy(
                            v_mine[:, t4, :, 0:D],
                            pv[:].rearrange("p (h d) -> p h d", d=D),
                        )

                    # --- allgather this half's k/v across the 4 cores
                    kv_in = dram.tile([KB + VB], f8, tag=f"kvin{half}")
                    nc.sync.dma_start(
                        kv_in[:KB].rearrange(
                            "(p h t) -> p h t", p=P, h=4, t=TOK
                        ),
                        k_mine[:],
                    )
                    nc.sync.dma_start(
                        kv_in[KB:].rearrange(
                            "(p a b c) -> p a b c", p=P, a=TC, b=8, c=VW
                        ),
                        v_mine[:],
                    )
                    kv_out = dram.tile([NR, KB + VB], f8, tag=f"kvout{half}")
                    nc.gpsimd.collective_compute(
                        "AllGather",
                        mybir.AluOpType.bypass,
                        replica_groups=groups,
                        ins=[kv_in.opt()],
                        outs=[kv_out.opt()],
                    )
                    kf = kvpool.tile(
                        [P, 4, NR, TOK], f8, tag=f"kf{half}",
                        name=f"k_full{half}",
                    )
                    vf = kvpool.tile(
                        [P, NR, TC, 8, VW], f8, tag=f"vf{half}",
                        name=f"v_full{half}",
                    )
                    for r in range(NR):
                        nc.sync.dma_start(
                            kf[:, :, r],
                            kv_out[r, :KB].rearrange(
                                "(p h t) -> p h t", p=P, h=4, t=TOK
                            ),
                        )
                        nc.sync.dma_start(
                            vf[:, r],
                            kv_out[r, KB:].rearrange(
                                "(p a b c) -> p a b c", p=P, a=TC, b=8, c=VW
                            ),
                        )
                    k_full[half] = kf
                    v_full[half] = vf

                y_sb = mid()

                # --- q projection + norm + rope (overlaps the collectives)
                q_sb = kvpool.tile([P, HH, TOK], bf16, tag="kq", name="q_sb")
                for g in range(2):
                    wq = wpool.tile([P, 4, KO, P], f8, tag="w1m", name="wq")
                    nc.sync.dma_start(
                        wq[:],
                        Wq_t[g * 4 : (g + 1) * 4].rearrange(
                            "g p ko m -> p g ko m"
                        ),
                    )
                    for j in range(4):
                        hh = g * 4 + j
                        pq = pp.tile([P, TOK], f32, tag="pp", name="pq")
                        dr_chain(
                            pq[:],
                            lambda c, j=j: wq[:, j, c : c + 2, :],
                            lambda c: y_sb[:, c : c + 2, :],
                        )
                        norm_rope_one(pq, mq, cosq_sb, sinq_sb, q_sb[:, hh])

                # --- attention: 2 heads share one exp; denominators ride in
                # row 64 of the px accumulators (ones column of v)
                xT = kvpool.tile([P, HH, TOK], bf16, tag="xTb", name="xT")
                den_stage = kvpool.tile(
                    [VW, H, TOK], bf16, tag="denst", name="den_stage"
                )
                den_sb = kvpool.tile([H, TOK], bf16, tag="den", name="den_sb")
                for hh in range(HH):
                    half, hl = hh // 4, hh % 4
                    kf, vf = k_full[half], v_full[half]
                    px = [
                        ps_x.tile([VW, TOK], f32, tag=f"px{i}", name=f"px{i}")
                        for i in range(2)
                    ]
                    for kc in range(H):  # 16 k-chunks of 128 tokens
                        r, tcl = kc // TC, kc % TC
                        ps = ps_s.tile([P, 2 * TOK], f32, tag="ps", name="ps")
                        for i in range(2):
                            off = i * D
                            nc.tensor.matmul(
                                ps[:, i * TOK : (i + 1) * TOK],
                                kf[
                                    off : off + D,
                                    hl,
                                    r,
                                    tcl * P : (tcl + 1) * P,
                                ],
                                q_sb[off : off + D, hh],
                                start=True,
                                stop=True,
                            )
                        prob = probp.tile(
                            [P, 2 * TOK], bf16, tag="prob", name="prob"
                        )
                        nc.scalar.activation(
                            prob[:], ps[:], AF.Exp, scale=1.0 / math.sqrt(D)
                        )
                        for i in range(2):
                            hl8 = hl * 2 + i
                            nc.tensor.matmul(
                                px[i][:],
                                vf[:, r, tcl, hl8],
                                prob[:, i * TOK : (i + 1) * TOK],
                                start=(kc == 0),
                                stop=(kc == H - 1),
                            )
                    for i in range(2):
                        h = hh * 2 + i
                        nc.vector.tensor_copy(
                            den_stage[D : D + 1, h], px[i][D : D + 1]
                        )
                        # 64-channel copy may retarget the other half-window
                        nc.vector.tensor_copy(
                            xT[i * D : (i + 1) * D, hh], px[i][0:D]
                        )
                nc.sync.dma_start(den_sb[:], den_stage[D : D + 1, :, :])

                # --- denominators: batched ln/exp + PE broadcast; the exp
                # bias folds in the 1/WS descale of V
                lnd = stat.tile([H, TOK], f32, tag="lnt", name="lnd")
                nc.scalar.activation(lnd[:], den_sb[:], AF.Ln)
                rsd = stat.tile([H, TOK], bf16, tag="rs", name="rsd")
                nc.scalar.activation(
                    rsd[:], lnd[:], AF.Exp, scale=-1.0, bias=lnws_sb[:]
                )
                xT8 = kvpool.tile([P, HH, TOK], f8, tag="xT8", name="xT8")
                for hh in range(HH):
                    bc = pp.tile([P, TOK], f32, tag="pp", name="dbc")
                    nc.tensor.matmul(
                        bc[:], selden_sb[:, hh], rsd[:], start=True, stop=True
                    )
                    nc.vector.tensor_mul(xT8[:, hh], xT[:, hh], bc[:])

                # --- Wo projection, accumulate into resid with 1/WS descale
                for g in range(2):
                    wo = wpool.tile([P, 4, KO, P], f8, tag="w1m", name="wo")
                    nc.sync.dma_start(
                        wo[:],
                        Wo_t[g * 4 : (g + 1) * 4].rearrange(
                            "g p ko m -> p g ko m"
                        ),
                    )
                    for j in range(4):
                        oc = g * 4 + j
                        po = pp.tile([P, TOK], f32, tag="pp", name="po")
                        dr_chain(
                            po[:],
                            lambda c, j=j: wo[:, j, c : c + 2, :],
                            lambda c: xT8[:, c : c + 2, :],
                        )
                        nc.vector.scalar_tensor_tensor(
                            resid[:, oc], po[:], 1.0 / WS, resid[:, oc],
                            op0=MUL, op1=ADD,
                        )

            # ================= cross-attention =================
            yT = sing.tile([P, KO, TOK], f8, name="yT")

            def ca_mid():
                rmsnorm_feat(resid, yT)
                return yT

            attention_block(
                srcT_sb, caWq, caWk, caWv, caWo,
                masks_sb["mq_ca"], masks_sb["mk_ca"], coskca_sb, sinkca_sb,
                ca_mid,
            )

            # ================= self-attention =================
            rmsnorm_feat(resid, yT)
            attention_block(
                yT, saWq, saWk, saWv, saWo,
                masks_sb["mq_sa"], masks_sb["mk_sa"], cosq_sb, sinq_sb,
                lambda: yT,
            )

            # ================= FFN =================
            rmsnorm_feat(resid, yT)
            hT = kvpool.tile([P, HC, TOK], f8, tag="hT", name="hT")
            for g in range(8):  # stream W1/W3 in eighths
                w1 = wpool.tile([P, 4, KO, P], f8, tag="w1m", name="w1")
                nc.sync.dma_start(
                    w1[:],
                    W1i[g * 4 : (g + 1) * 4].rearrange("g p ko m -> p g ko m"),
                )
                w3 = wpool.tile([P, 4, KO, P], f8, tag="w1m", name="w3")
                nc.sync.dma_start(
                    w3[:],
                    W3i[g * 4 : (g + 1) * 4].rearrange("g p ko m -> p g ko m"),
                )
                for j in range(4):
                    hc = g * 4 + j
                    p1 = pp.tile([P, TOK], f32, tag="pp", name="p1")
                    dr_chain(
                        p1[:],
                        lambda c, j=j: w1[:, j, c : c + 2, :],
                        lambda c: yT[:, c : c + 2, :],
                    )
                    p3 = pp.tile([P, TOK], f32, tag="pp", name="p3")
                    dr_chain(
                        p3[:],
                        lambda c, j=j: w3[:, j, c : c + 2, :],
                        lambda c: yT[:, c : c + 2, :],
                    )
                    s1 = stat.tile([P, TOK], f32, tag="raw", name="s1")
                    nc.scalar.activation(s1[:], p1[:], AF.Silu, scale=1.0 / WS)
                    nc.vector.scalar_tensor_tensor(
                        hT[:, hc], p3[:], 1.0 / WS, s1[:], op0=MUL, op1=MUL
                    )
            for oc in range(KO):
                w2 = w2pool.tile([P, HC, P], f8, tag="w2", name="w2")
                nc.sync.dma_start(w2[:], W2i[oc])
                po = pp.tile([P, TOK], f32, tag="pp", name="po2")
                for h2 in range(HC // 2):
                    nc.tensor.matmul(
                        po[:],
                        w2[:, 2 * h2 : 2 * h2 + 2, :],
                        hT[:, 2 * h2 : 2 * h2 + 2, :],
                        start=(h2 == 0),
                        stop=(h2 == HC // 2 - 1),
                        perf_mode=DRM,
                    )
                nc.vector.scalar_tensor_tensor(
                    resid[:, oc], po[:], 1.0 / WS, resid[:, oc],
                    op0=MUL, op1=ADD,
                )
                nc.sync.dma_start(outT[:, oc], resid[:, oc])

    _split_multiwait(nc)
    return nc


def _prep_inputs(inputs):
    """Full problem inputs -> list of 8 per-core in_maps."""
    tgt = np.asarray(inputs["tgt"], np.float32)
    src = np.asarray(inputs["src"], np.float32)
    tgt_pos = np.asarray(inputs["tgt_pos"], np.int32)
    src_pos = np.asarray(inputs["src_pos"], np.int32)

    pre_ca_w = np.asarray(inputs["pre_ca_w"], np.float32)
    pre_sa_w = np.asarray(inputs["pre_sa_w"], np.float32)
    pre_ffn_w = np.asarray(inputs["pre_ffn_w"], np.float32)

    def fold(Wname, w):
        return np.asarray(inputs[Wname], np.float32) * w[:, None]

    ca_Wq = fold("ca_Wq", pre_ca_w)
    ca_Wkv = np.asarray(inputs["ca_Wkv"], np.float32)
    ca_Wk, ca_Wv = ca_Wkv[:, :DIM], ca_Wkv[:, DIM:]
    ca_Wo = np.asarray(inputs["ca_Wo"], np.float32)
    sa_Wq = fold("sa_Wq", pre_sa_w)
    sa_Wkv = fold("sa_Wkv", pre_sa_w)
    sa_Wk, sa_Wv = sa_Wkv[:, :DIM], sa_Wkv[:, DIM:]
    sa_Wo = np.asarray(inputs["sa_Wo"], np.float32)
    W1 = fold("W1", pre_ffn_w)
    W3 = fold("W3", pre_ffn_w)
    W2 = np.asarray(inputs["W2"], np.float32)

    shared = {
        "caWq": _lhsT_f8(ca_Wq),
        "caWk": _lhsT_f8(ca_Wk),
        "caWv": _rhs_f8(ca_Wv),
        "caWo": _lhsT_f8(ca_Wo),
        "saWq": _lhsT_f8(sa_Wq),
        "saWk": _lhsT_f8(sa_Wk),
        "saWv": _rhs_f8(sa_Wv),
        "saWo": _lhsT_f8(sa_Wo),
        "W1": _lhsT_f8(W1),
        "W3": _lhsT_f8(W3),
        "W2": _lhsT_f8(W2),
    }

    blk2 = np.zeros((P, 2), BF)
    blk2[:D, 0] = 1
    blk2[D:, 1] = 1
    shared["blk2"] = blk2

    def head_mask(w):  # [2, 128] with per-head norm weight
        m = np.zeros((2, P), np.float32)
        m[0, :D] = w
        m[1, D:] = w
        return m.astype(BF).copy()

    shared["mq_ca"] = head_mask(np.asarray(inputs["ca_qn"], np.float32))
    shared["mk_ca"] = head_mask(np.asarray(inputs["ca_kn"], np.float32))
    shared["mq_sa"] = head_mask(np.asarray(inputs["sa_qn"], np.float32))
    shared["mk_sa"] = head_mask(np.asarray(inputs["sa_kn"], np.float32))

    r64 = np.zeros((D, D), np.float32)
    half = D // 2
    for j in range(half):
        r64[j, j + half] = -1.0  # rot[j] = -x[j+32]
        r64[j + half, j] = 1.0  # rot[j+32] = x[j]
    rt = r64.T  # lhsT (matmul computes lhsT.T @ rhs)
    rotm = np.zeros((P, P), np.float32)
    rotm[:D, :D] = rt
    rotm[D:, D:] = rt
    shared["rotm"] = rotm.astype(BF).copy()

    shared["ones_c"] = np.ones((P, 1), BF)
    shared["ones_r128"] = np.ones((1, P), BF)

    seld = np.zeros((H, HH, P), np.float32)
    for hh in range(HH):
        seld[2 * hh, hh, :D] = 1
        seld[2 * hh + 1, hh, D:] = 1
    shared["selden"] = seld.astype(BF).copy()

    in_maps = []
    for c in range(NCORES):
        s, part = c // NR, c % NR
        rows = slice(part * TOK, (part + 1) * TOK)
        m = dict(shared)
        m["tgtT"] = _featmajor(tgt[s, rows])
        m["srcTb"] = _featmajor(src[s, rows]).astype(F8)
        cq, sq_ = _rope_tables(tgt_pos[s, rows])
        ck, sk = _rope_tables(src_pos[s, rows])
        m["cosq"], m["sinq"] = cq, sq_
        m["coskca"], m["sinkca"] = ck, sk
        in_maps.append(m)
    return in_maps


def _get_nc():
    if "nc" not in _cache:
        _cache["nc"] = _build_bass()
    return _cache["nc"]


def run(inputs, trace=False):
    """Run on 8 cores; returns (full_output, exec_time_ns_or_None)."""
    if trace:
        _install_ntff_hook()
    from concourse.bass_utils import run_bass_kernel_spmd

    in_maps = _prep_inputs(inputs)
    nc = _get_nc()
    res = run_bass_kernel_spmd(
        nc, in_maps, core_ids=list(range(NCORES)), trace=trace
    )
    out = np.empty((B, N, DIM), np.float32)
    for c in range(NCORES):
        s, part = c // NR, c % NR
        arr = np.asarray(res.results[c]["outT"])  # [128, 8, TOK]
        rows = slice(part * TOK, (part + 1) * TOK)
        out[s, rows] = np.transpose(arr, (2, 1, 0)).reshape(TOK, DIM)
    return out, res.exec_time_ns


def kernel(**inputs):
    out, _ = run(inputs, trace=False)
    return out
